# revision 6
# baseline (speedup 1.0000x reference)
"""Trainium2 Bass kernel for AngularSymmetryMod — v3.

Per core (2 molecules): partitions = (b:2, i:32, half:2) = 128, free = 264
packed (j<=k) pairs. out[b,i,l] = sum_pairs ang*rad*cut with the 40-value
parameter grid collapsed to 20 reductions (4 angular quadrant fields x 5
radial gaussians), assembled into 40 columns by the final matmul.

v3 structure (from v1/v2 trace analysis):
 - fp32 everywhere: this DVE build runs bf16 tensor_tensor at HALF fp32 rate
   (custom uop tables), so bf16 only lost precision.
 - Radial chain lives entirely on ScalarE: sq_r = Square(q*0.5*sqrt(eta) -
   sqrt(eta)*Rs) (5 ops) then ONE wide Exp over [P,1320]. Square+Exp share
   the exp_and_others table; only Sin switches tables (2 loads, first hidden
   in launch).
 - GpSimd (tensor_tensor only - no TensorScalarPtr support) computes q, den,
   cut and the y/z dot products from SBUF.
 - cut is folded into the radial weights (Wc5 = rad5 * cut, one wide op)
   during the load2 window -> off the post-sin tail.
 - Post-sin tail: Sin -> (1+-x)^2 -> 4th powers on ACT while DVE runs the
   20 fused multiply+accumulate reductions back-to-back.
 - DMA: 5 transfers with triggers spread over sync/scalar/gpsimd queues
   (each DIRECT2D costs ~0.65us serially per queue), ordered so the radial
   inputs (uff) land first and geo-z last.
"""

import sys
import numpy as np
import ml_dtypes

sys.path.insert(0, "/opt/trn_rl_repo")


def _to_bf16(a):
    return np.asarray(a, dtype=np.float32).astype(ml_dtypes.bfloat16)

from contextlib import ExitStack

import concourse.bass as bass
import concourse.tile as tile
from concourse import bacc, mybir
from concourse.bass_utils import run_bass_kernel_spmd

B, N, L = 16, 32, 40
NCORES = 8
B_LOC = B // NCORES  # 2
P = 128
NT = 264
NOFF = 248

BOHR = 0.52917721092
ITA = 1.12
RS_VALS = np.array([0.5, 1.17, 1.83, 2.5, 3.17]) / BOHR
NR, NM = 5, 4
SQ_ETA = float(np.sqrt(ITA))
TWO_PI = float(2.0 * np.pi)
RC = float(12582912.0)

F32 = mybir.dt.float32
OP = mybir.AluOpType
ACT = mybir.ActivationFunctionType


def _pair_index():
    pairs = [(j, k) for j in range(N) for k in range(j + 1, N)]  # 496
    halves = [pairs[0::2], pairs[1::2]]
    tri_j = np.zeros((2, NT), dtype=np.int64)
    tri_k = np.zeros((2, NT), dtype=np.int64)
    for h in range(2):
        for t, (j, k) in enumerate(halves[h]):
            tri_j[h, t], tri_k[h, t] = j, k
        for t2, j in enumerate(range(h * 16, (h + 1) * 16)):
            tri_j[h, NOFF + t2] = tri_k[h, NOFF + t2] = j
    return tri_j, tri_k


_TRI_J, _TRI_K = _pair_index()


def _build():
    nc = bacc.Bacc("TRN2", target_bir_lowering=False, debug=False)
    uu_d = nc.declare_dram_parameter("uu", [P, 2 * NT], F32, isOutput=False)
    ff_d = nc.declare_dram_parameter("ff", [P, 2 * NT], F32, isOutput=False)
    # pes rows (q=0..9): 0-3 bsel for hi-srcs, 4-7 bsel for lo-srcs,
    # 8/9 = -ci hi/lo. cols: 3 lhsT blocks [128] then 6 rhs blocks [264]
    # (per coord: vj-rhs with ones rows, ck-rhs with zero rows)
    BF = mybir.dt.bfloat16
    pes_d = nc.declare_dram_parameter("pes", [10, 3 * 128 + 6 * NT], BF, isOutput=False)
    cc_d = nc.declare_dram_parameter("cc", [P, 67], F32, isOutput=False)
    out_d = nc.declare_dram_parameter("out", [B_LOC * N, L], F32, isOutput=True)

    with tile.TileContext(nc) as tc, ExitStack() as ctx:
        pool = ctx.enter_context(tc.tile_pool(name="sb", bufs=1))
        scr_pool = ctx.enter_context(tc.tile_pool(name="scr", bufs=8))
        psum = ctx.enter_context(tc.tile_pool(name="ps", bufs=1, space="PSUM"))

        # ---- input DMAs ----
        BF = mybir.dt.bfloat16
        uu = pool.tile([P, 2 * NT], F32, name="uu", tag="uu")
        ff = pool.tile([P, 2 * NT], F32, name="ff", tag="ff")
        pes = pool.tile([10, 3 * 128 + 6 * NT], BF, name="pes", tag="pes")
        cc = pool.tile([P, 67], F32, name="cc", tag="cc")
        nc.sync.dma_start(uu[:], uu_d[:])
        nc.sync.dma_start(pes[:], pes_d[:])
        nc.sync.dma_start(ff[:], ff_d[:])
        nc.gpsimd.dma_start(cc[:], cc_d[:])
        # constant tiles (gpsimd memsets, after its DMA triggers)
        rsb = pool.tile([P, NR], F32, name="rsb", tag="rsb")
        for r in range(NR):
            nc.gpsimd.memset(rsb[:, r : r + 1], float(-SQ_ETA * RS_VALS[r]))
        wcol = pool.tile([P, NT], F32, name="wcol", tag="wcol")
        nc.gpsimd.memset(wcol[:, 0:NOFF], 0.25)
        nc.gpsimd.memset(wcol[:, NOFF:NT], 0.125)
        # (a) dummy first activation with a launch-time-ready input so the
        # exp_and_others ACT_TABLE_LOAD is hoisted into the launch window
        dmy = pool.tile([P, 1], F32, name="dmy", tag="dmy")
        nc.scalar.activation(dmy[:], rsb[:, 0:1], ACT.Square)
        uj = uu[:, 0:NT]
        uk = uu[:, NT : 2 * NT]
        fj = ff[:, 0:NT]
        fk = ff[:, NT : 2 * NT]
        cic = cc[:, 0:3]
        cst = cc[:, 3:67]

        # ---- geo expansion on the TensorEngine ----
        # vj = bsel.T@cj(hi+lo) - ci (ones rhs rows x -ci lhsT rows); ck plain.
        # One [10,128] lhsT per coordinate, exact fp32 via hi/lo bf16.
        vj_ps, ck_ps = [], []
        for c in range(3):
            lhsT = pes[:, c * 128 : (c + 1) * 128]
            r0 = 3 * 128 + 2 * c * NT
            gv = psum.tile([P, NT], F32, name=f"vjp{c}", tag=f"vjp{c}")
            gk = psum.tile([P, NT], F32, name=f"ckp{c}", tag=f"ckp{c}")
            nc.tensor.matmul(gv[:], lhsT, pes[:, r0 : r0 + NT])
            nc.tensor.matmul(gk[:], lhsT, pes[:, r0 + NT : r0 + 2 * NT])
            vj_ps.append(gv)
            ck_ps.append(gk)

        # ---- q on DVE (gates the whole ACT radial chain); den/cut on GpSimd ----
        q = pool.tile([P, NT], F32, name="q", tag="q")
        den = pool.tile([P, NT], F32, name="den", tag="den")
        nc.vector.tensor_tensor(q[:], uj, uk, OP.add)
        nc.gpsimd.tensor_tensor(den[:], uj, uk, OP.mult)
        cutw = pool.tile([P, NT], F32, name="cutw", tag="cutw")
        cut = pool.tile([P, NT], F32, name="cut", tag="cut")
        nc.gpsimd.tensor_tensor(cutw[:], fj, fk, OP.mult)
        nc.gpsimd.tensor_tensor(cut[:], cutw[:], wcol[:], OP.mult)

        # ---- radial: 5 biased Squares + one wide Exp, all on ACT ----
        sq5 = pool.tile([P, 5 * NT], F32, name="sq5", tag="sq5")
        rad5 = pool.tile([P, 5 * NT], F32, name="rad5", tag="rad5")
        for r in range(NR):
            nc.scalar.activation(
                sq5[:, r * NT : (r + 1) * NT], q[:], ACT.Square,
                bias=rsb[:, r : r + 1], scale=0.5 * SQ_ETA)
        nc.scalar.activation(rad5[:], sq5[:], ACT.Exp, scale=-1.0)

        # ---- DVE dot-product chain: vk = ck - ci, p_c = vk * vj(PSUM) ----
        pcs = []
        for c in range(3):
            vk = pool.tile([P, NT], F32, name=f"vk{c}", tag=f"vk{c}")
            nc.vector.tensor_scalar(
                vk[:], ck_ps[c][:], cic[:, c : c + 1], None, OP.subtract)
            pc = pool.tile([P, NT], F32, name=f"p{c}", tag=f"p{c}")
            nc.vector.scalar_tensor_tensor(
                pc[:], vk[:], 0.0, vj_ps[c][:], OP.bypass, OP.mult)
            pcs.append(pc)
        px, py, pz = pcs

        # denb = 2pi*(den + 1e-5); rden = 1/denb   (DVE, fits in gaps)
        denb = pool.tile([P, NT], F32, name="denb", tag="denb")
        rden = pool.tile([P, NT], F32, name="rden", tag="rden")
        nc.vector.tensor_scalar(denb[:], den[:], 1e-5, TWO_PI, OP.add, OP.mult)
        nc.vector.reciprocal_approx_fast(rden[:], denb[:])

        dot01 = pool.tile([P, NT], F32, name="dot01", tag="dot01")
        dot = pool.tile([P, NT], F32, name="dot", tag="dot")
        nc.vector.tensor_tensor(dot01[:], px[:], py[:], OP.add)
        nc.vector.tensor_tensor(dot[:], dot01[:], pz[:], OP.add)

        tp2 = pool.tile([P, 2 * NT], F32, name="tp2", tag="tp2")
        nc.vector.tensor_tensor(tp2[:, 0:NT], dot[:], rden[:], OP.mult)
        nc.vector.tensor_scalar(tp2[:, NT : 2 * NT], tp2[:, 0:NT], 0.25, None, OP.add)
        nf2 = pool.tile([P, 2 * NT], F32, name="nf2", tag="nf2")
        nc.vector.tensor_scalar(nf2[:], tp2[:], RC, RC, OP.add, OP.subtract)
        fr2 = pool.tile([P, 2 * NT], F32, name="fr2", tag="fr2")
        nc.vector.tensor_tensor(fr2[:, 0:NT], tp2[:, 0:NT], nf2[:, 0:NT], OP.subtract)
        nc.vector.tensor_tensor(fr2[:, NT:], tp2[:, NT:], nf2[:, NT:], OP.subtract)

        # ---- sin/cos + angular powers, split per trig half so the +s reduce
        # block starts while the cos half is still on ACT ----
        one = pool.tile([P, 1], F32, name="one", tag="one")
        nc.gpsimd.memset(one[:], 1.0)
        cs = pool.tile([P, 2 * NT], F32, name="cs", tag="cs")
        up = pool.tile([P, 2 * NT], F32, name="up", tag="up")
        ap = pool.tile([P, 2 * NT], F32, name="ap", tag="ap")
        um = pool.tile([P, 2 * NT], F32, name="um", tag="um")
        am = pool.tile([P, 2 * NT], F32, name="am", tag="am")
        Hs, Hc = slice(0, NT), slice(NT, 2 * NT)
        nc.scalar.activation(cs[:, Hs], fr2[:, Hs], ACT.Sin, scale=TWO_PI)
        # s-half powers on DVE while ACT runs the cos-half Sin
        nc.vector.tensor_scalar(up[:, Hs], cs[:, Hs], 1.0, None, OP.add)
        nc.vector.tensor_tensor(ap[:, Hs], up[:, Hs], up[:, Hs], OP.mult)
        nc.vector.tensor_tensor(ap[:, Hs], ap[:, Hs], ap[:, Hs], OP.mult)
        nc.scalar.activation(cs[:, Hc], fr2[:, Hc], ACT.Sin, scale=TWO_PI)
        nc.scalar.activation(up[:, Hc], cs[:, Hc], ACT.Square, bias=one[:], scale=1.0)
        nc.scalar.activation(ap[:, Hc], up[:, Hc], ACT.Square)
        for h in (Hs, Hc):
            nc.scalar.activation(um[:, h], cs[:, h], ACT.Square, bias=one[:], scale=-1.0)
            nc.scalar.activation(am[:, h], um[:, h], ACT.Square)

        # ---- Wc5 = rad5 * cut on DVE (GpSimd shares the SBUF port with DVE,
        # so putting these there stalls the fr2 chain instead) ----
        Wc5 = pool.tile([P, 5 * NT], F32, name="Wc5", tag="Wc5")
        for r in range(NR):
            nc.vector.tensor_tensor(
                Wc5[:, r * NT : (r + 1) * NT], rad5[:, r * NT : (r + 1) * NT],
                cut[:], OP.mult)


        # ---- 20 fused multiply+reduce on DVE; final matmuls overlap the train.
        # spart split into +side/-side tiles: tile-granular dependency tracking
        # lets the +side matmuls fire as soon as their 10 reduces finish ----
        spart_p = pool.tile([P, NR * 2], F32, name="spart_p", tag="spart_p")
        spart_m = pool.tile([P, NR * 2], F32, name="spart_m", tag="spart_m")
        s2p = psum.tile([64, L], F32, name="s2p", tag="s2p")
        spp = spart_p[:].rearrange("p (r t) -> p r t", r=NR, t=2)
        spm = spart_m[:].rearrange("p (r t) -> p r t", r=NR, t=2)
        o3 = s2p[:].rearrange("n (g r t) -> n g r t", g=2, r=NR, t=NM)

        # (field, col-offset within [s|c], dest-tile, tcol within tile):
        # +s -> spart_p col 1, +c -> spart_p col 0, -s -> spart_m 1, -c -> spart_m 0
        def reduce_block(field, off, dst, tc2):
            for r in range(NR):
                scr = scr_pool.tile([P, NT], F32, name=f"scr{r}{tc2}", tag="scr")
                nc.vector.scalar_tensor_tensor(
                    scr[:], Wc5[:, r * NT : (r + 1) * NT], 0.0,
                    field[:, off : off + NT], OP.bypass, OP.mult,
                    accum_out=dst[:, r * 2 + tc2 : r * 2 + tc2 + 1])

        reduce_block(ap, 0, spart_p, 1)    # +s
        reduce_block(ap, NT, spart_p, 0)   # +c
        nc.tensor.matmul(o3[:, 0, :, 0:2], cst, spp[:, :, 0:2])
        nc.tensor.matmul(o3[:, 1, :, 2:4], cst, spp[:, :, 0:2])
        s2s = pool.tile([64, L], F32, name="s2s", tag="s2s")
        o3s = s2s[:].rearrange("n (g r t) -> n g r t", g=2, r=NR, t=NM)
        reduce_block(am, 0, spart_m, 1)    # -s
        nc.vector.tensor_copy(o3s[:, 0, :, 0:2], o3[:, 0, :, 0:2])
        reduce_block(am, NT, spart_m, 0)   # -c
        nc.vector.tensor_copy(o3s[:, 1, :, 2:4], o3[:, 1, :, 2:4])
        nc.tensor.matmul(o3[:, 0, :, 2:4], cst, spm[:, :, 0:2])
        nc.tensor.matmul(o3[:, 1, :, 0:2], cst, spm[:, :, 0:2])
        nc.vector.tensor_copy(o3s[:, 0, :, 2:4], o3[:, 0, :, 2:4])
        nc.vector.tensor_copy(o3s[:, 1, :, 0:2], o3[:, 1, :, 0:2])
        nc.sync.dma_start(out_d[:], s2s[:])

    nc.compile()
    return nc


def _ensure_ntff_hook():
    import types

    try:
        from antenv.axon_hooks import get_axon_ntff_profile_hook
        if get_axon_ntff_profile_hook() is not None:
            return
        have_mod = True
    except ImportError:
        have_mod = False
    try:
        if "/root/.axon_site" not in sys.path:
            sys.path.insert(0, "/root/.axon_site")
        from trn_agent_boot.trn_boot import _ntff_profile_via_ctypes

        hook = _ntff_profile_via_ctypes("/opt/axon/libaxon_pjrt.so")
        if hook is None:
            return
    except Exception:
        return
    if have_mod:
        from antenv import axon_hooks
        axon_hooks.set_axon_ntff_profile_hook(hook)
    else:
        m = types.ModuleType("antenv.axon_hooks")
        _h = [hook]
        m.get_axon_ntff_profile_hook = lambda: _h[0]
        m.set_axon_ntff_profile_hook = lambda h: _h.__setitem__(0, h)
        import antenv
        antenv.axon_hooks = m
        sys.modules["antenv.axon_hooks"] = m


_NC = None


def _get_nc():
    global _NC
    if _NC is None:
        _NC = _build()
    return _NC


_CST = None


def _const_blob():
    global _CST
    if _CST is None:
        _CST = np.repeat(np.eye(64, dtype=np.float32), 2, axis=0)
    return _CST


def _host_pack(d_cutoff, d, atom_coordinates):
    """Pure gather/replication of raw inputs into the per-core layouts."""
    d_cutoff = np.ascontiguousarray(d_cutoff, dtype=np.float32)
    d = np.ascontiguousarray(d, dtype=np.float32)
    coords = np.ascontiguousarray(atom_coordinates, dtype=np.float32)

    p = np.arange(P)
    b_of_p = p // (N * 2)
    i_of_p = (p // 2) % N
    half = p % 2
    jt = _TRI_J[half]  # [P, NT]
    kt = _TRI_K[half]

    in_maps = []
    for c in range(NCORES):
        cd = coords[c * B_LOC : (c + 1) * B_LOC]
        dd = d[c * B_LOC : (c + 1) * B_LOC]
        fc = d_cutoff[c * B_LOC : (c + 1) * B_LOC]
        uu = np.empty((P, 2 * NT), dtype=np.float32)
        uu[:, 0:NT] = dd[b_of_p[:, None], i_of_p[:, None], jt]
        uu[:, NT : 2 * NT] = dd[b_of_p[:, None], i_of_p[:, None], kt]
        ffb = np.empty((P, 2 * NT), dtype=np.float32)
        ffb[:, 0:NT] = fc[b_of_p[:, None], i_of_p[:, None], jt]
        ffb[:, NT : 2 * NT] = fc[b_of_p[:, None], i_of_p[:, None], kt]
        # pes: rows 0-3 bsel (hi), 4-7 bsel (lo), 8/9 -ci hi/lo.
        # cols: 3 lhsT blocks [128] then per coord (vj-rhs | ck-rhs) [264] each.
        pes = np.zeros((10, 3 * 128 + 6 * NT), dtype=np.float32)
        row_of_p = b_of_p * 2 + half  # [P]
        ci_full = cd[b_of_p, i_of_p]  # [P, 3]
        for cc in range(3):
            lb = cc * 128
            pes[row_of_p, lb + np.arange(P)] = 1.0
            pes[row_of_p + 4, lb + np.arange(P)] = 1.0
            ci_hi = _to_bf16(-ci_full[:, cc]).astype(np.float32)
            ci_lo = _to_bf16(-ci_full[:, cc] - ci_hi).astype(np.float32)
            pes[8, lb : lb + P] = ci_hi
            pes[9, lb : lb + P] = ci_lo
            for bb in range(B_LOC):
                for h in range(2):
                    row = bb * 2 + h
                    for jk, tri in ((0, _TRI_J), (1, _TRI_K)):
                        src = cd[bb, tri[h], cc]  # [NT]
                        hi16 = _to_bf16(src).astype(np.float32)
                        lo16 = _to_bf16(src - hi16).astype(np.float32)
                        cbase = 3 * 128 + (2 * cc + jk) * NT
                        pes[row, cbase : cbase + NT] = hi16
                        pes[row + 4, cbase : cbase + NT] = lo16
            # vj-rhs ones rows; ck-rhs zero rows
            vbase = 3 * 128 + 2 * cc * NT
            pes[8, vbase : vbase + NT] = 1.0
            pes[9, vbase : vbase + NT] = 1.0
        pes_bf = _to_bf16(pes)
        ccb = np.empty((P, 67), dtype=np.float32)
        ccb[:, 0:3] = cd[b_of_p, i_of_p]
        ccb[:, 3:67] = _const_blob()
        in_maps.append({"uu": uu, "ff": ffb, "pes": pes_bf, "cc": ccb})
    return in_maps


def kernel(d_cutoff, d, atom_coordinates, _trace=False):
    if _trace:
        _ensure_ntff_hook()
    nc = _get_nc()
    in_maps = _host_pack(d_cutoff, d, atom_coordinates)
    res = run_bass_kernel_spmd(nc, in_maps, core_ids=list(range(NCORES)), trace=_trace)
    out = np.concatenate(
        [res.results[c]["out"].reshape(B_LOC, N, L) for c in range(NCORES)], axis=0
    ).astype(np.float32)
    if _trace:
        kernel._last_results = res
    return out


# revision 8
# speedup vs baseline: 1.0568x; 1.0568x over previous
"""Trainium2 Bass kernel for AngularSymmetryMod (final).

Per core (2 molecules): partitions = (b:2, i:32, half:2) = 128, free = 264
packed (j<=k) pairs. out[b,i,l] = sum_pairs ang*rad*cut with the 40-value
parameter grid collapsed to 20 fused multiply+reduce ops (4 angular quadrant
fields x 5 radial gaussians), assembled into 40 columns by final matmuls.

Structure (each point trace-verified on hardware):
 - fp32 compute throughout: this DVE build runs 16-bit tensor_tensor at or
   below the fp32 rate (custom uop tables), so narrow dtypes only lose bits.
 - Pair coordinates are not DMA'd: the TensorEngine expands them from a 13KB
   source via exact hi/lo-bf16 matmuls, with the -ci subtraction folded into
   the vj matmuls (ones-rows in the rhs against -ci rows in the lhsT).
 - Radial chain entirely on ScalarE (5 biased Squares + one wide Exp);
   Square+Exp share the exp_and_others table, Sin is the only table switch.
   A dummy first activation hoists the first table load into the launch
   window; the second load hides behind the DVE dot-product chain.
 - GpSimd runs only den/cut (it shares its SBUF port with the Vector engine,
   so anything on it during DVE-critical stretches stalls the DVE).
 - DMA triggers: uu first (gates the radial chain), then pes, ff on the sync
   queue; descriptor queues are FIFO in trigger order across all engines.
 - Post-sin: per-half Sin so the +s reduce block starts early; Wc = rad*cut
   folded during the trig-table-load window; spart and the PSUM output are
   split per side so the final matmuls/copies overlap the reduce train.
"""

import sys
import numpy as np
import ml_dtypes

sys.path.insert(0, "/opt/trn_rl_repo")


def _to_bf16(a):
    return np.asarray(a, dtype=np.float32).astype(ml_dtypes.bfloat16)

from contextlib import ExitStack

import concourse.bass as bass
import concourse.tile as tile
from concourse import bacc, mybir
from concourse.bass_utils import run_bass_kernel_spmd

B, N, L = 16, 32, 40
NCORES = 8
B_LOC = B // NCORES  # 2
P = 128
NT = 264
NOFF = 248

BOHR = 0.52917721092
ITA = 1.12
RS_VALS = np.array([0.5, 1.17, 1.83, 2.5, 3.17]) / BOHR
NR, NM = 5, 4
SQ_ETA = float(np.sqrt(ITA))
TWO_PI = float(2.0 * np.pi)
RC = float(12582912.0)

F32 = mybir.dt.float32
OP = mybir.AluOpType
ACT = mybir.ActivationFunctionType


def _pair_index():
    pairs = [(j, k) for j in range(N) for k in range(j + 1, N)]  # 496
    halves = [pairs[0::2], pairs[1::2]]
    tri_j = np.zeros((2, NT), dtype=np.int64)
    tri_k = np.zeros((2, NT), dtype=np.int64)
    for h in range(2):
        for t, (j, k) in enumerate(halves[h]):
            tri_j[h, t], tri_k[h, t] = j, k
        for t2, j in enumerate(range(h * 16, (h + 1) * 16)):
            tri_j[h, NOFF + t2] = tri_k[h, NOFF + t2] = j
    return tri_j, tri_k


_TRI_J, _TRI_K = _pair_index()


def _build():
    nc = bacc.Bacc("TRN2", target_bir_lowering=False, debug=False)
    uu_d = nc.declare_dram_parameter("uu", [P, 2 * NT], F32, isOutput=False)
    ff_d = nc.declare_dram_parameter("ff", [P, 2 * NT], F32, isOutput=False)
    # pes rows (q=0..9): 0-3 bsel for hi-srcs, 4-7 bsel for lo-srcs,
    # 8/9 = -ci hi/lo. cols: 3 lhsT blocks [128] then 6 rhs blocks [264]
    # (per coord: vj-rhs with ones rows, ck-rhs with zero rows)
    BF = mybir.dt.bfloat16
    pes_d = nc.declare_dram_parameter("pes", [10, 3 * 128 + 6 * NT], BF, isOutput=False)
    cc_d = nc.declare_dram_parameter("cc", [P, 67], F32, isOutput=False)
    out_d = nc.declare_dram_parameter("out", [B_LOC * N, L], F32, isOutput=True)

    with tile.TileContext(nc) as tc, ExitStack() as ctx:
        pool = ctx.enter_context(tc.tile_pool(name="sb", bufs=1))
        scr_pool = ctx.enter_context(tc.tile_pool(name="scr", bufs=8))
        psum = ctx.enter_context(tc.tile_pool(name="ps", bufs=1, space="PSUM"))

        # ---- input DMAs ----
        BF = mybir.dt.bfloat16
        uu = pool.tile([P, 2 * NT], F32, name="uu", tag="uu")
        ff = pool.tile([P, 2 * NT], F32, name="ff", tag="ff")
        pes = pool.tile([10, 3 * 128 + 6 * NT], BF, name="pes", tag="pes")
        cc = pool.tile([P, 67], F32, name="cc", tag="cc")
        nc.sync.dma_start(uu[:], uu_d[:])
        nc.sync.dma_start(pes[:], pes_d[:])
        nc.sync.dma_start(ff[:], ff_d[:])
        nc.gpsimd.dma_start(cc[:], cc_d[:])
        # constant tiles (gpsimd memsets, after its DMA triggers)
        rsb = pool.tile([P, NR], F32, name="rsb", tag="rsb")
        for r in range(NR):
            nc.gpsimd.memset(rsb[:, r : r + 1], float(-SQ_ETA * RS_VALS[r]))
        wcol = pool.tile([P, NT], F32, name="wcol", tag="wcol")
        nc.gpsimd.memset(wcol[:, 0:NOFF], 0.25)
        nc.gpsimd.memset(wcol[:, NOFF:NT], 0.125)
        # (a) dummy first activation with a launch-time-ready input so the
        # exp_and_others ACT_TABLE_LOAD is hoisted into the launch window
        dmy = pool.tile([P, 1], F32, name="dmy", tag="dmy")
        nc.scalar.activation(dmy[:], rsb[:, 0:1], ACT.Square)
        uj = uu[:, 0:NT]
        uk = uu[:, NT : 2 * NT]
        fj = ff[:, 0:NT]
        fk = ff[:, NT : 2 * NT]
        cic = cc[:, 0:3]
        cst = cc[:, 3:67]

        # ---- geo expansion on the TensorEngine ----
        # vj = bsel.T@cj(hi+lo) - ci (ones rhs rows x -ci lhsT rows); ck plain.
        # One [10,128] lhsT per coordinate, exact fp32 via hi/lo bf16.
        vj_ps, ck_ps = [], []
        for c in range(3):
            lhsT = pes[:, c * 128 : (c + 1) * 128]
            r0 = 3 * 128 + 2 * c * NT
            gv = psum.tile([P, NT], F32, name=f"vjp{c}", tag=f"vjp{c}")
            gk = psum.tile([P, NT], F32, name=f"ckp{c}", tag=f"ckp{c}")
            nc.tensor.matmul(gv[:], lhsT, pes[:, r0 : r0 + NT])
            nc.tensor.matmul(gk[:], lhsT, pes[:, r0 + NT : r0 + 2 * NT])
            vj_ps.append(gv)
            ck_ps.append(gk)

        # ---- q on DVE (gates the whole ACT radial chain); den/cut on GpSimd ----
        q = pool.tile([P, NT], F32, name="q", tag="q")
        den = pool.tile([P, NT], F32, name="den", tag="den")
        nc.vector.tensor_tensor(q[:], uj, uk, OP.add)
        nc.gpsimd.tensor_tensor(den[:], uj, uk, OP.mult)
        cutw = pool.tile([P, NT], F32, name="cutw", tag="cutw")
        cut = pool.tile([P, NT], F32, name="cut", tag="cut")
        nc.gpsimd.tensor_tensor(cutw[:], fj, fk, OP.mult)
        nc.gpsimd.tensor_tensor(cut[:], cutw[:], wcol[:], OP.mult)

        # ---- radial: 5 biased Squares + one wide Exp, all on ACT ----
        sq5 = pool.tile([P, 5 * NT], F32, name="sq5", tag="sq5")
        rad5 = pool.tile([P, 5 * NT], F32, name="rad5", tag="rad5")
        for r in range(NR):
            nc.scalar.activation(
                sq5[:, r * NT : (r + 1) * NT], q[:], ACT.Square,
                bias=rsb[:, r : r + 1], scale=0.5 * SQ_ETA)
        nc.scalar.activation(rad5[:], sq5[:], ACT.Exp, scale=-1.0)

        # ---- DVE dot-product chain: vk = ck - ci, p_c = vk * vj(PSUM) ----
        pcs = []
        for c in range(3):
            vk = pool.tile([P, NT], F32, name=f"vk{c}", tag=f"vk{c}")
            nc.vector.tensor_scalar(
                vk[:], ck_ps[c][:], cic[:, c : c + 1], None, OP.subtract)
            pc = pool.tile([P, NT], F32, name=f"p{c}", tag=f"p{c}")
            nc.vector.scalar_tensor_tensor(
                pc[:], vk[:], 0.0, vj_ps[c][:], OP.bypass, OP.mult)
            pcs.append(pc)
        px, py, pz = pcs

        # denb = 2pi*(den + 1e-5); rden = 1/denb   (DVE, fits in gaps)
        denb = pool.tile([P, NT], F32, name="denb", tag="denb")
        rden = pool.tile([P, NT], F32, name="rden", tag="rden")
        nc.vector.tensor_scalar(denb[:], den[:], 1e-5, TWO_PI, OP.add, OP.mult)
        nc.vector.reciprocal_approx_fast(rden[:], denb[:])

        dot01 = pool.tile([P, NT], F32, name="dot01", tag="dot01")
        dot = pool.tile([P, NT], F32, name="dot", tag="dot")
        nc.vector.tensor_tensor(dot01[:], px[:], py[:], OP.add)
        nc.vector.tensor_tensor(dot[:], dot01[:], pz[:], OP.add)

        tp2 = pool.tile([P, 2 * NT], F32, name="tp2", tag="tp2")
        nc.vector.tensor_tensor(tp2[:, 0:NT], dot[:], rden[:], OP.mult)
        nc.vector.tensor_scalar(tp2[:, NT : 2 * NT], tp2[:, 0:NT], 0.25, None, OP.add)
        nf2 = pool.tile([P, 2 * NT], F32, name="nf2", tag="nf2")
        nc.vector.tensor_scalar(nf2[:], tp2[:], RC, RC, OP.add, OP.subtract)
        fr2 = pool.tile([P, 2 * NT], F32, name="fr2", tag="fr2")
        nc.vector.tensor_tensor(fr2[:, 0:NT], tp2[:, 0:NT], nf2[:, 0:NT], OP.subtract)
        nc.vector.tensor_tensor(fr2[:, NT:], tp2[:, NT:], nf2[:, NT:], OP.subtract)

        # ---- sin/cos + angular powers, split per trig half so the +s reduce
        # block starts while the cos half is still on ACT ----
        one = pool.tile([P, 1], F32, name="one", tag="one")
        nc.gpsimd.memset(one[:], 1.0)
        cs = pool.tile([P, 2 * NT], F32, name="cs", tag="cs")
        up = pool.tile([P, 2 * NT], F32, name="up", tag="up")
        ap = pool.tile([P, 2 * NT], F32, name="ap", tag="ap")
        um = pool.tile([P, 2 * NT], F32, name="um", tag="um")
        am = pool.tile([P, 2 * NT], F32, name="am", tag="am")
        Hs, Hc = slice(0, NT), slice(NT, 2 * NT)
        nc.scalar.activation(cs[:, Hs], fr2[:, Hs], ACT.Sin, scale=TWO_PI)
        # s-half powers on DVE while ACT runs the cos-half Sin
        nc.vector.tensor_scalar(up[:, Hs], cs[:, Hs], 1.0, None, OP.add)
        nc.vector.tensor_tensor(ap[:, Hs], up[:, Hs], up[:, Hs], OP.mult)
        nc.vector.tensor_tensor(ap[:, Hs], ap[:, Hs], ap[:, Hs], OP.mult)
        nc.scalar.activation(cs[:, Hc], fr2[:, Hc], ACT.Sin, scale=TWO_PI)
        nc.scalar.activation(up[:, Hc], cs[:, Hc], ACT.Square, bias=one[:], scale=1.0)
        nc.scalar.activation(ap[:, Hc], up[:, Hc], ACT.Square)
        for h in (Hs, Hc):
            nc.scalar.activation(um[:, h], cs[:, h], ACT.Square, bias=one[:], scale=-1.0)
            nc.scalar.activation(am[:, h], um[:, h], ACT.Square)

        # ---- Wc5 = rad5 * cut on DVE (GpSimd shares the SBUF port with DVE,
        # so putting these there stalls the fr2 chain instead) ----
        Wc5 = pool.tile([P, 5 * NT], F32, name="Wc5", tag="Wc5")
        for r in range(NR):
            nc.vector.tensor_tensor(
                Wc5[:, r * NT : (r + 1) * NT], rad5[:, r * NT : (r + 1) * NT],
                cut[:], OP.mult)


        # ---- 20 fused multiply+reduce on DVE; final matmuls overlap the train.
        # spart split into +side/-side tiles: tile-granular dependency tracking
        # lets the +side matmuls fire as soon as their 10 reduces finish ----
        spart_p = pool.tile([P, NR * 2], F32, name="spart_p", tag="spart_p")
        spart_m = pool.tile([P, NR * 2], F32, name="spart_m", tag="spart_m")
        spp = spart_p[:].rearrange("p (r t) -> p r t", r=NR, t=2)
        spm = spart_m[:].rearrange("p (r t) -> p r t", r=NR, t=2)

        # (field, col-offset within [s|c], dest-tile, tcol within tile):
        # +s -> spart_p col 1, +c -> spart_p col 0, -s -> spart_m 1, -c -> spart_m 0
        def reduce_block(field, off, dst, tc2):
            for r in range(NR):
                scr = scr_pool.tile([P, NT], F32, name=f"scr{r}{tc2}", tag="scr")
                nc.vector.scalar_tensor_tensor(
                    scr[:], Wc5[:, r * NT : (r + 1) * NT], 0.0,
                    field[:, off : off + NT], OP.bypass, OP.mult,
                    accum_out=dst[:, r * 2 + tc2 : r * 2 + tc2 + 1])

        # separate PSUM tiles per side so the +side copies only depend on the
        # +side matmuls (tile-granular dependency tracking)
        s2a = psum.tile([64, 20], F32, name="s2a", tag="s2a")
        s2b = psum.tile([64, 20], F32, name="s2b", tag="s2b")
        a3 = s2a[:].rearrange("n (g r t) -> n g r t", g=2, r=NR, t=2)
        b3 = s2b[:].rearrange("n (g r t) -> n g r t", g=2, r=NR, t=2)
        reduce_block(ap, 0, spart_p, 1)    # +s
        reduce_block(ap, NT, spart_p, 0)   # +c
        nc.tensor.matmul(a3[:, 0], cst, spp[:, :, 0:2])
        nc.tensor.matmul(a3[:, 1], cst, spp[:, :, 0:2])
        s2s = pool.tile([64, L], F32, name="s2s", tag="s2s")
        o3s = s2s[:].rearrange("n (g r t) -> n g r t", g=2, r=NR, t=NM)
        reduce_block(am, 0, spart_m, 1)    # -s
        nc.vector.tensor_copy(o3s[:, 0, :, 0:2], a3[:, 0])
        reduce_block(am, NT, spart_m, 0)   # -c
        nc.vector.tensor_copy(o3s[:, 1, :, 2:4], a3[:, 1])
        nc.tensor.matmul(b3[:, 0], cst, spm[:, :, 0:2])
        nc.tensor.matmul(b3[:, 1], cst, spm[:, :, 0:2])
        nc.vector.tensor_copy(o3s[:, 0, :, 2:4], b3[:, 0])
        nc.vector.tensor_copy(o3s[:, 1, :, 0:2], b3[:, 1])
        nc.sync.dma_start(out_d[:], s2s[:])

    nc.compile()
    return nc


def _ensure_ntff_hook():
    import types

    try:
        from antenv.axon_hooks import get_axon_ntff_profile_hook
        if get_axon_ntff_profile_hook() is not None:
            return
        have_mod = True
    except ImportError:
        have_mod = False
    try:
        if "/root/.axon_site" not in sys.path:
            sys.path.insert(0, "/root/.axon_site")
        from trn_agent_boot.trn_boot import _ntff_profile_via_ctypes

        hook = _ntff_profile_via_ctypes("/opt/axon/libaxon_pjrt.so")
        if hook is None:
            return
    except Exception:
        return
    if have_mod:
        from antenv import axon_hooks
        axon_hooks.set_axon_ntff_profile_hook(hook)
    else:
        m = types.ModuleType("antenv.axon_hooks")
        _h = [hook]
        m.get_axon_ntff_profile_hook = lambda: _h[0]
        m.set_axon_ntff_profile_hook = lambda h: _h.__setitem__(0, h)
        import antenv
        antenv.axon_hooks = m
        sys.modules["antenv.axon_hooks"] = m


_NC = None


def _get_nc():
    global _NC
    if _NC is None:
        _NC = _build()
    return _NC


_CST = None


def _const_blob():
    global _CST
    if _CST is None:
        _CST = np.repeat(np.eye(64, dtype=np.float32), 2, axis=0)
    return _CST


def _host_pack(d_cutoff, d, atom_coordinates):
    """Pure gather/replication of raw inputs into the per-core layouts."""
    d_cutoff = np.ascontiguousarray(d_cutoff, dtype=np.float32)
    d = np.ascontiguousarray(d, dtype=np.float32)
    coords = np.ascontiguousarray(atom_coordinates, dtype=np.float32)

    p = np.arange(P)
    b_of_p = p // (N * 2)
    i_of_p = (p // 2) % N
    half = p % 2
    jt = _TRI_J[half]  # [P, NT]
    kt = _TRI_K[half]

    in_maps = []
    for c in range(NCORES):
        cd = coords[c * B_LOC : (c + 1) * B_LOC]
        dd = d[c * B_LOC : (c + 1) * B_LOC]
        fc = d_cutoff[c * B_LOC : (c + 1) * B_LOC]
        uu = np.empty((P, 2 * NT), dtype=np.float32)
        uu[:, 0:NT] = dd[b_of_p[:, None], i_of_p[:, None], jt]
        uu[:, NT : 2 * NT] = dd[b_of_p[:, None], i_of_p[:, None], kt]
        ffb = np.empty((P, 2 * NT), dtype=np.float32)
        ffb[:, 0:NT] = fc[b_of_p[:, None], i_of_p[:, None], jt]
        ffb[:, NT : 2 * NT] = fc[b_of_p[:, None], i_of_p[:, None], kt]
        # pes: rows 0-3 bsel (hi), 4-7 bsel (lo), 8/9 -ci hi/lo.
        # cols: 3 lhsT blocks [128] then per coord (vj-rhs | ck-rhs) [264] each.
        pes = np.zeros((10, 3 * 128 + 6 * NT), dtype=np.float32)
        row_of_p = b_of_p * 2 + half  # [P]
        ci_full = cd[b_of_p, i_of_p]  # [P, 3]
        for cc in range(3):
            lb = cc * 128
            pes[row_of_p, lb + np.arange(P)] = 1.0
            pes[row_of_p + 4, lb + np.arange(P)] = 1.0
            ci_hi = _to_bf16(-ci_full[:, cc]).astype(np.float32)
            ci_lo = _to_bf16(-ci_full[:, cc] - ci_hi).astype(np.float32)
            pes[8, lb : lb + P] = ci_hi
            pes[9, lb : lb + P] = ci_lo
            for bb in range(B_LOC):
                for h in range(2):
                    row = bb * 2 + h
                    for jk, tri in ((0, _TRI_J), (1, _TRI_K)):
                        src = cd[bb, tri[h], cc]  # [NT]
                        hi16 = _to_bf16(src).astype(np.float32)
                        lo16 = _to_bf16(src - hi16).astype(np.float32)
                        cbase = 3 * 128 + (2 * cc + jk) * NT
                        pes[row, cbase : cbase + NT] = hi16
                        pes[row + 4, cbase : cbase + NT] = lo16
            # vj-rhs ones rows; ck-rhs zero rows
            vbase = 3 * 128 + 2 * cc * NT
            pes[8, vbase : vbase + NT] = 1.0
            pes[9, vbase : vbase + NT] = 1.0
        pes_bf = _to_bf16(pes)
        ccb = np.empty((P, 67), dtype=np.float32)
        ccb[:, 0:3] = cd[b_of_p, i_of_p]
        ccb[:, 3:67] = _const_blob()
        in_maps.append({"uu": uu, "ff": ffb, "pes": pes_bf, "cc": ccb})
    return in_maps


def kernel(d_cutoff, d, atom_coordinates, _trace=False):
    if _trace:
        _ensure_ntff_hook()
    nc = _get_nc()
    in_maps = _host_pack(d_cutoff, d, atom_coordinates)
    res = run_bass_kernel_spmd(nc, in_maps, core_ids=list(range(NCORES)), trace=_trace)
    out = np.concatenate(
        [res.results[c]["out"].reshape(B_LOC, N, L) for c in range(NCORES)], axis=0
    ).astype(np.float32)
    if _trace:
        kernel._last_results = res
    return out
